# revision 1
# baseline (speedup 1.0000x reference)
"""AudioAttentionPooler Trainium2 kernel.

Algorithm (algebraically identical to the reference, ~60x fewer FLOPs):
  scores[b,t,h] = x[b,t,:] @ Wq[:,h]        Wq = fold(query*scale, kv_w_k)  [C,h]
  (k-bias shifts scores uniformly along t -> softmax-invariant -> dropped)
  e = exp(scores)                           (mask folded into x and Z instead)
  Z[b,h] = sum_t e[b,t,h] * mask[b,t]
  px[b,h,:] = sum_t e[b,t,h] * (mask[b,t] * x[b,t,:])   (pool BEFORE v-proj)
  out1[b,h*64+d] = (px[b,h,:] @ Wv[:,h*64+d]) / Z[b,h]
  out = out1 @ out_w + (kv_b_v @ out_w + out_b)   (v-bias exact: attn sums to 1)

Sharding: data-parallel over batch, 4 batch elements per core x 8 cores.
x is fed in both [T,C] and [C,T] layouts (host transpose) because the PE
contracts over the partition dim: scores contract over C, pooling over T.
"""

import numpy as np
import ml_dtypes

BF16 = ml_dtypes.bfloat16

HIDDEN = 1024
NH = 16
HD = 64
PROJ = 1024
B, T = 32, 2048
NCORES = 8
NB = B // NCORES          # 4 batch elems per core
KT = HIDDEN // 128        # 8 C-tiles
MT = T // 128             # 16 T-chunks
F8 = ml_dtypes.float8_e4m3
F8MAX = 240.0             # conservative e4m3 range cap

_CACHED_NC = None


def _build_nc(reps=1):
    import concourse.bacc as bacc
    import concourse.mybir as mybir
    import concourse.tile as tile

    f32 = mybir.dt.float32
    bf16 = mybir.dt.bfloat16
    f8 = mybir.dt.float8e4

    nc = bacc.Bacc("TRN2", target_bir_lowering=False, debug=False)

    x_d = nc.dram_tensor("x", [NB, T, HIDDEN], bf16, kind="ExternalInput")
    xt_d = nc.dram_tensor("xt", [NB, 128, MT, KT, 128], f8, kind="ExternalInput")
    wq_d = nc.dram_tensor("wq", [128, KT, NH], f8, kind="ExternalInput")
    wv_d = nc.dram_tensor("wv", [128, KT, NH, HD], bf16, kind="ExternalInput")
    wo_d = nc.dram_tensor("wo", [128, KT, 2, 512], bf16, kind="ExternalInput")
    mcol_d = nc.dram_tensor("mcol", [128, NB, MT], bf16, kind="ExternalInput")
    biasrep_d = nc.dram_tensor("biasrep", [NB, PROJ], f32, kind="ExternalInput")
    onescol_d = nc.dram_tensor("onescol", [1, 128], f32, kind="ExternalInput")
    idf_d = nc.dram_tensor("idf", [128, 128], f32, kind="ExternalInput")
    escale_d = nc.dram_tensor("escale", [128, 1], f32, kind="ExternalInput")
    out_d = nc.dram_tensor("out", [NB, PROJ], f32, kind="ExternalOutput")

    from contextlib import nullcontext

    with tile.TileContext(nc) as tc:
        with (
            tc.tile_pool(name="consts", bufs=1) as consts,
            tc.tile_pool(name="xpool", bufs=3) as xpool,
            tc.tile_pool(name="xtpool", bufs=3) as xtpool,
            tc.tile_pool(name="work", bufs=3) as work,
            tc.tile_pool(name="small", bufs=1) as small,
            tc.tile_pool(name="scps", bufs=2, space="PSUM") as scps,
            tc.tile_pool(name="pxps", bufs=2, space="PSUM") as pxps,
            tc.tile_pool(name="tps", bufs=2, space="PSUM") as tps,
            tc.tile_pool(name="bigps", bufs=1, space="PSUM") as bigps,
        ):
            wq_sb = consts.tile([128, KT, NH], f8)
            wv_sb = consts.tile([128, KT, NH, HD], bf16)
            wo_sb = consts.tile([128, KT, 2, 512], bf16)
            mcol_sb = consts.tile([128, NB, MT], bf16)
            biasrep_sb = consts.tile([NB, PROJ], f32)
            onescol_sb = consts.tile([1, 128], f32)
            idf_sb = consts.tile([128, 128], f32)
            escale_sb = consts.tile([128, 1], f32)
            nc.sync.dma_start(wq_sb[:], wq_d[:])

            # persistent accumulators across the b-loop
            pxall_sb = small.tile([128, KT, NH, NB], bf16)

            rep_ctx = tc.For_i(0, reps, 1) if reps > 1 else nullcontext()
            with rep_ctx:
              for b in range(NB):
                  x_sb = xpool.tile([128, MT, HIDDEN], bf16)
                  xt_sb = xtpool.tile([128, MT, KT, 128], f8)
                  for m4 in range(4):
                      nc.sync.dma_start(
                          xt_sb[:, m4 * 4:(m4 + 1) * 4],
                          xt_d[b, :, m4 * 4:(m4 + 1) * 4],
                      )
                  for m4 in range(4):
                      nc.sync.dma_start(
                          x_sb[:, m4 * 4:(m4 + 1) * 4],
                          x_d[b, m4 * 512:(m4 + 1) * 512].rearrange(
                              "(m p) c -> p m c", p=128
                          ),
                      )
                  # deferred const loads, ordered by first use so early DMA
                  # bandwidth goes to the batch data stream; stage-3/4 weights
                  # stream per-k AFTER all batch data so the pooling loop is
                  # never delayed and stage 3/4 chase the weight chunks
                  if b == 0:
                      nc.sync.dma_start(escale_sb[:], escale_d[:])
                      nc.sync.dma_start(onescol_sb[:], onescol_d[:])
                      nc.sync.dma_start(mcol_sb[:], mcol_d[:])
                      nc.sync.dma_start(idf_sb[:], idf_d[:])
                  elif b == NB - 1:
                      for k in range(KT):
                          nc.sync.dma_start(wv_sb[:, k], wv_d[:, k])
                      for k in range(KT):
                          nc.sync.dma_start(wo_sb[:, k], wo_d[:, k])
                      nc.sync.dma_start(biasrep_sb[:], biasrep_d[:])

                  # --- scores[t, h] = x @ Wq ---------------------------------
                  sc_sb = work.tile([128, MT, NH], f32)
                  for m2 in range(MT // 4):
                      sc_ps = scps.tile([128, 4, NH], f32, tag="sc")
                      for m4 in range(4):
                          m = m2 * 4 + m4
                          for k in range(KT):
                              nc.tensor.matmul(
                                  sc_ps[:, m4, :],
                                  xt_sb[:, m, k, :],
                                  wq_sb[:, k, :],
                                  start=(k == 0),
                                  stop=(k == KT - 1),
                              )
                      nc.vector.tensor_copy(sc_sb[:, m2 * 4:(m2 + 1) * 4, :], sc_ps[:])

                  # --- e = exp(scores) (bf16); mask is folded into x and the
                  # Z moving operand, so no explicit mask multiply is needed.
                  # Split into per-group ops so the pooling matmuls can trail
                  # the score stream instead of waiting for all 16 chunks -----
                  e_sb = work.tile([128, MT, NH], bf16)
                  for m2 in range(MT // 4):
                      nc.scalar.activation(
                          e_sb[:, m2 * 4:(m2 + 1) * 4, :],
                          sc_sb[:, m2 * 4:(m2 + 1) * 4, :],
                          mybir.ActivationFunctionType.Exp,
                          scale=escale_sb[:],
                      )

                  # --- Z[h] = sum_t e (output oriented [NH, 1]) --------------
                  z_ps = tps.tile([NH, 1], f32, tag="tps")
                  for m in range(MT):
                      nc.tensor.matmul(
                          z_ps[:],
                          e_sb[:, m, :],
                          mcol_sb[:, b, m:m + 1],
                          start=(m == 0),
                          stop=(m == MT - 1),
                      )
                  z_sb = work.tile([NH, 1], f32)
                  nc.vector.tensor_copy(z_sb[:], z_ps[:])
                  # broadcast 1/Z down all 128 partitions: [NH,1] -T-> [1,NH]
                  # -K=1 matmul-> [128,NH] -reciprocal-> sbuf
                  zt_ps = tps.tile([1, NH], f32, tag="tps")
                  nc.tensor.transpose(zt_ps[:], z_sb[:], idf_sb[0:NH, 0:NH])
                  zt_sb = work.tile([1, NH], f32)
                  nc.vector.tensor_copy(zt_sb[:], zt_ps[:])
                  zbc_ps = tps.tile([128, NH], f32, tag="tps")
                  nc.tensor.matmul(
                      zbc_ps[:], onescol_sb[:], zt_sb[:], start=True, stop=True
                  )
                  zinv_sb = work.tile([128, NH], f32)
                  nc.vector.reciprocal(zinv_sb[:], zbc_ps[:])

                  # --- px[h, c] = e.T @ x (unnormalized pool) ----------------
                  px_sb = work.tile([NH, HIDDEN], f32)
                  for c2 in range(2):
                      px_ps = pxps.tile([NH, 512], f32, tag="px")
                      for m in range(MT):
                          nc.tensor.matmul(
                              px_ps[:],
                              e_sb[:, m, :],
                              x_sb[:, m, c2 * 512:(c2 + 1) * 512],
                              start=(m == 0),
                              stop=(m == MT - 1),
                          )
                      nc.vector.tensor_copy(px_sb[:, c2 * 512:(c2 + 1) * 512], px_ps[:])

                  # --- pxT: [C-tile, h] with b packed in the free dim --------
                  for k in range(KT):
                      pxt_ps = tps.tile([128, NH], f32, tag="tps")
                      nc.tensor.transpose(
                          pxt_ps[:], px_sb[:, k * 128:(k + 1) * 128], idf_sb[0:NH, 0:NH]
                      )
                      nc.vector.tensor_mul(pxall_sb[:, k, :, b], pxt_ps[:], zinv_sb[:])

              # --- stage 3: out1_raw[b, hd] = px @ Wv -------------------------
              out1_ps = bigps.tile([NB, HIDDEN], f32)
              for h in range(NH):
                  for k in range(KT):
                      nc.tensor.matmul(
                          out1_ps[:, h * HD:(h + 1) * HD],
                          pxall_sb[:, k, h, :],
                          wv_sb[:, k, h, :],
                          start=(k == 0),
                          stop=(k == KT - 1),
                      )

              # --- out1T: [hd-tile, b] (out1 already normalized; per-k copies
              # so the copy/transpose/stage-4 chain trails stage 3 head-by-head
              # instead of waiting for the full [4,1024] psum) -----------------
              out1n_sb = small.tile([NB, HIDDEN], f32)
              o1t_sb = small.tile([128, KT, NB], bf16)
              for k in range(KT):
                  nc.vector.tensor_copy(
                      out1n_sb[:, k * 128:(k + 1) * 128],
                      out1_ps[:, k * 128:(k + 1) * 128],
                  )
                  o1t_ps = tps.tile([128, NB], f32, tag="tps")
                  nc.tensor.transpose(
                      o1t_ps[:], out1n_sb[:, k * 128:(k + 1) * 128], idf_sb[0:NB, 0:NB]
                  )
                  nc.vector.tensor_copy(o1t_sb[:, k, :], o1t_ps[:])

              # --- stage 4: out = out1 @ out_w + bias -------------------------
              of_sb = small.tile([NB, PROJ], f32)
              of_ps0 = scps.tile([NB, 512], f32, tag="sc")
              of_ps1 = scps.tile([NB, 512], f32, tag="sc")
              for k in range(KT):
                  for p2, of_ps in ((0, of_ps0), (1, of_ps1)):
                      nc.tensor.matmul(
                          of_ps[:],
                          o1t_sb[:, k, :],
                          wo_sb[:, k, p2, :],
                          start=(k == 0),
                          stop=(k == KT - 1),
                      )
              for p2, of_ps in ((0, of_ps0), (1, of_ps1)):
                  nc.vector.tensor_add(
                      of_sb[:, p2 * 512:(p2 + 1) * 512],
                      of_ps[:],
                      biasrep_sb[:, p2 * 512:(p2 + 1) * 512],
                  )
              nc.sync.dma_start(out_d[:], of_sb[:])

    nc.compile()
    return nc


def _get_nc():
    global _CACHED_NC
    if _CACHED_NC is None:
        _CACHED_NC = _build_nc()
    return _CACHED_NC


def _prep_inputs(hidden_states, mask, kv_w, kv_b, out_w, out_b, query):
    """Host-side sharding + weight preprocessing -> per-core input maps."""
    x = np.ascontiguousarray(hidden_states, dtype=np.float32)
    mask = np.asarray(mask)
    kv_w = np.asarray(kv_w, dtype=np.float32)
    kv_b = np.asarray(kv_b, dtype=np.float32)
    out_w = np.asarray(out_w, dtype=np.float32)
    out_b = np.asarray(out_b, dtype=np.float32)
    query = np.asarray(query, dtype=np.float32)

    scale = 1.0 / HD ** 0.5
    Wk = kv_w[:, :HIDDEN]
    Wv = kv_w[:, HIDDEN:]
    qh = query.reshape(NH, HD)
    # fold query into the k-projection: Wq[c, h]
    Wq = np.einsum("chd,hd->ch", Wk.reshape(HIDDEN, NH, HD), qh) * scale
    bias_final = kv_b[HIDDEN:] @ out_w + out_b  # v-bias is exact post-pool

    # dynamic power-of-2 fp8 scales (exactly unwound inside the exp activation)
    sw = 2.0 ** np.floor(np.log2(F8MAX / max(np.abs(Wq).max(), 1e-30)))
    sx = 2.0 ** np.floor(np.log2(F8MAX / max(np.abs(x).max(), 1e-30)))
    sx = min(sx, 1.0)
    escale = np.full((128, 1), 1.0 / (sw * sx), np.float32)
    wq_r = np.ascontiguousarray(
        (Wq * sw).reshape(KT, 128, NH).transpose(1, 0, 2)
    ).astype(F8)  # [128, KT, NH], fp8 with exp-unwound scale
    wv_r = np.ascontiguousarray(
        Wv.reshape(KT, 128, NH, HD).transpose(1, 0, 2, 3)
    ).astype(BF16)  # [128, KT, NH, HD]
    wo_r = np.ascontiguousarray(
        out_w.reshape(KT, 128, 2, 512).transpose(1, 0, 2, 3)
    ).astype(BF16)  # [128, KT, 2, 512]
    onescol = np.ones((1, 128), np.float32)
    idf = np.eye(128, dtype=np.float32)

    mvalid = (mask != 0).astype(np.float32)      # reference masks where mask == 0
    x_bf = (x * mvalid[:, :, None]).astype(BF16)  # pre-masked pooling copy [B, T, C]
    # xt chunked layout: xtr[b, p, m, k, t] = x[b, m*128+t, k*128+p]; per-partition
    # rows are contiguous in (m, k, t) so m-range DMA slices stay order-aligned
    xt_bf = np.ascontiguousarray(
        (x * sx).reshape(B, MT, 128, KT, 128).transpose(0, 4, 1, 3, 2)
    ).astype(F8)

    in_maps = []
    for c in range(NCORES):
        sl = slice(c * NB, (c + 1) * NB)
        # mcol[p, b, m] = valid(mask[b, m*128+p])
        mcol = np.ascontiguousarray(
            mvalid[sl].reshape(NB, MT, 128).transpose(2, 0, 1)
        ).astype(BF16)
        in_maps.append({
            "x": x_bf[sl],
            "xt": xt_bf[sl],
            "wq": wq_r,
            "wv": wv_r,
            "wo": wo_r,
            "mcol": mcol,
            "biasrep": np.ascontiguousarray(
                np.broadcast_to(bias_final[None, :], (NB, PROJ))
            ),
            "onescol": onescol,
            "idf": idf,
            "escale": escale,
        })
    return in_maps


def kernel(hidden_states, mask, kv_w, kv_b, out_w, out_b, query, **_unused):
    from concourse.bass_utils import run_bass_kernel_spmd

    nc = _get_nc()
    in_maps = _prep_inputs(hidden_states, mask, kv_w, kv_b, out_w, out_b, query)
    res = run_bass_kernel_spmd(nc, in_maps, list(range(NCORES)))
    out = np.concatenate([res.results[i]["out"] for i in range(NCORES)], axis=0)
    return out.astype(np.float32)



# revision 4
# speedup vs baseline: 1.1564x; 1.1564x over previous
"""AudioAttentionPooler Trainium2 kernel (v2).

Algorithm (algebraically identical to the reference, ~60x fewer FLOPs):
  scores[b,t,h] = x[b,t,:] @ Wq[:,h]        Wq = fold(query*scale, kv_w_k)  [C,h]
  (k-bias shifts scores uniformly along t -> softmax-invariant -> dropped)
  e'' = 16*exp(scores)  (exp input-scale unwinds the fp8 scales)
  e'  = e'' - 15        (fp8; w = (15 + e')/16 up to the global 1/16Z norm)
  Z16[b,h] = sum_t (15 + e'[b,t,h]) * mask[b,t]       (= 16*Z)
  px'[b,h,:] = sum_t e'[b,t,h] * xp[b,t,:]            (pool BEFORE v-proj)
  u'[b,:]    = sum_t 15 * xp[b,t,:]                   (u-column of the same matmul)
  pxn[b,h,:] = (px' + u') / (Z16 * sp)                (xp carries scale sp)
  out1[b,h*64+d] = pxn[b,h,:] @ Wv[:,h*64+d]
  out = out1 @ out_w + (kv_b_v @ out_w + out_b)   (v-bias exact: attn sums to 1)

v2 changes vs v1:
  - pooling copy of x is fp8 with host-side error diffusion along t
    (quantization carries cancel in the near-uniform attention sum),
    halving the dominant DMA stream (16.8MB bf16 -> 8.4MB fp8 per core)
  - pooling matmuls use fp8 DoubleRow perf mode (2 t-tiles per
    instruction, 2x moving-operand rate), with the attention weights as
    the 17-column stationary (16 heads + constant-15 u column)
  - Z comes from tiny DoubleRow matmuls against a mask column

Sharding: data-parallel over batch, 4 batch elements per core x 8 cores.
"""

import numpy as np
import ml_dtypes

BF16 = ml_dtypes.bfloat16
F8 = ml_dtypes.float8_e4m3

HIDDEN = 1024
NH = 16
HD = 64
PROJ = 1024
B, T = 32, 2048
NCORES = 8
NB = B // NCORES          # 4 batch elems per core
KT = HIDDEN // 128        # 8 C-tiles
MT = T // 128             # 16 T-chunks
M2 = MT // 2              # 8 DoubleRow pairs
F8MAX = 240.0             # conservative e4m3 range cap
LN16 = float(np.log(16.0))

_CACHED_NC = None


def _build_nc(reps=1):
    import concourse.bacc as bacc
    import concourse.mybir as mybir
    import concourse.tile as tile

    f32 = mybir.dt.float32
    bf16 = mybir.dt.bfloat16
    f8 = mybir.dt.float8e4
    DR = mybir.MatmulPerfMode.DoubleRow

    nc = bacc.Bacc("TRN2", target_bir_lowering=False, debug=False)

    xt_d = nc.dram_tensor("xt", [NB, 128, MT, KT, 128], f8, kind="ExternalInput")
    xp_d = nc.dram_tensor("xp", [NB, 128, MT, HIDDEN], f8, kind="ExternalInput")
    wq_d = nc.dram_tensor("wq", [128, KT, NH], f8, kind="ExternalInput")
    wv_d = nc.dram_tensor("wv", [128, KT, NH, HD], bf16, kind="ExternalInput")
    wo_d = nc.dram_tensor("wo", [128, KT, 2, 512], bf16, kind="ExternalInput")
    mcol_d = nc.dram_tensor("mcol", [128, NB, MT, 16], f8, kind="ExternalInput")
    biasrep_d = nc.dram_tensor("biasrep", [NB, PROJ], f32, kind="ExternalInput")
    onescol_d = nc.dram_tensor("onescol", [1, 128], f32, kind="ExternalInput")
    idf_d = nc.dram_tensor("idf", [128, 128], f32, kind="ExternalInput")
    escale_d = nc.dram_tensor("escale", [128, 1], f32, kind="ExternalInput")
    ebias_d = nc.dram_tensor("ebias", [128, 1], f32, kind="ExternalInput")
    znv_d = nc.dram_tensor("znv", [1, NB], f32, kind="ExternalInput")
    spv_d = nc.dram_tensor("spv", [1, 1], f32, kind="ExternalInput")
    out_d = nc.dram_tensor("out", [NB, PROJ], f32, kind="ExternalOutput")

    from contextlib import nullcontext

    with tile.TileContext(nc) as tc:
        with (
            tc.tile_pool(name="consts", bufs=1) as consts,
            tc.tile_pool(name="xppool", bufs=3) as xppool,
            tc.tile_pool(name="xtpool", bufs=3) as xtpool,
            tc.tile_pool(name="work", bufs=3) as work,
            tc.tile_pool(name="small", bufs=1) as small,
            tc.tile_pool(name="scps", bufs=2, space="PSUM") as scps,
            tc.tile_pool(name="pxps", bufs=2, space="PSUM") as pxps,
            tc.tile_pool(name="tps", bufs=2, space="PSUM") as tps,
            tc.tile_pool(name="bigps", bufs=1, space="PSUM") as bigps,
        ):
            wq_sb = consts.tile([128, KT, NH], f8)
            wv_sb = consts.tile([128, KT, NH, HD], bf16)
            wo_sb = consts.tile([128, KT, 2, 512], bf16)
            mcol_sb = consts.tile([128, NB, MT, 16], f8)
            biasrep_sb = consts.tile([NB, PROJ], f32)
            onescol_sb = consts.tile([1, 128], f32)
            idf_sb = consts.tile([128, 128], f32)
            escale_sb = consts.tile([128, 1], f32)
            ebias_sb = consts.tile([128, 1], f32)
            znv_sb = consts.tile([1, NB], f32)
            spv_sb = consts.tile([1, 1], f32)
            nc.sync.dma_start(wq_sb[:], wq_d[:])

            # persistent accumulators across the b-loop
            pxall_sb = small.tile([128, KT, NH, NB], bf16)

            rep_ctx = tc.For_i(0, reps, 1) if reps > 1 else nullcontext()
            with rep_ctx:
              for b in range(NB):
                  xt_sb = xtpool.tile([128, MT, KT, 128], f8)
                  xp_sb = xppool.tile([128, MT, HIDDEN], f8)
                  for m4 in range(4):
                      nc.sync.dma_start(
                          xt_sb[:, m4 * 4:(m4 + 1) * 4],
                          xt_d[b, :, m4 * 4:(m4 + 1) * 4],
                      )
                  for m4 in range(4):
                      nc.sync.dma_start(
                          xp_sb[:, m4 * 4:(m4 + 1) * 4],
                          xp_d[b, :, m4 * 4:(m4 + 1) * 4],
                      )
                  # deferred const loads: small consts after the first batch
                  # data stream, big stage-3/4 weights after ALL batch data
                  if b == 0:
                      nc.sync.dma_start(escale_sb[:], escale_d[:])
                      nc.sync.dma_start(ebias_sb[:], ebias_d[:])
                      nc.sync.dma_start(onescol_sb[:], onescol_d[:])
                      nc.sync.dma_start(znv_sb[:], znv_d[:])
                      nc.sync.dma_start(spv_sb[:], spv_d[:])
                      nc.sync.dma_start(mcol_sb[:], mcol_d[:])
                      nc.sync.dma_start(idf_sb[:], idf_d[:])
                  elif b == NB - 1:
                      for k in range(KT):
                          nc.sync.dma_start(wv_sb[:, k], wv_d[:, k])
                      for k in range(KT):
                          nc.sync.dma_start(wo_sb[:, k], wo_d[:, k])
                      nc.sync.dma_start(biasrep_sb[:], biasrep_d[:])

                  # --- scores[t, h] = x @ Wq; e' = 16*exp - 15 (fp8) ---------
                  e_sb = work.tile([128, MT, 32], f8)
                  nc.vector.memset(e_sb[:, :, 16:17], 15.0)
                  for m2 in range(MT // 4):
                      sc_ps = scps.tile([128, 4, NH], f32, tag="sc")
                      for m4 in range(4):
                          m = m2 * 4 + m4
                          for k in range(KT):
                              nc.tensor.matmul(
                                  sc_ps[:, m4, :],
                                  xt_sb[:, m, k, :],
                                  wq_sb[:, k, :],
                                  start=(k == 0),
                                  stop=(k == KT - 1),
                              )
                      ebig_sb = work.tile([128, 4, NH], f32, tag="ebig")
                      nc.scalar.activation(
                          ebig_sb[:],
                          sc_ps[:],
                          mybir.ActivationFunctionType.Exp,
                          bias=ebias_sb[:],
                          scale=escale_sb[:],
                      )
                      nc.vector.tensor_scalar_sub(
                          e_sb[:, m2 * 4:(m2 + 1) * 4, 0:16], ebig_sb[:], 15.0
                      )

                  # --- Z16[h] = sum_t (15 + e') * mask -----------------------
                  z_ps = tps.tile([17, 1], f32, tag="tps")
                  for m2 in range(M2):
                      nc.tensor.matmul(
                          z_ps[:],
                          e_sb[:, 2 * m2:2 * m2 + 2, 0:17],
                          mcol_sb[:, b, 2 * m2:2 * m2 + 2, 0:1],
                          perf_mode=DR,
                          start=(m2 == 0),
                          stop=(m2 == M2 - 1),
                      )
                  z_sb = work.tile([17, 1], f32)
                  nc.vector.tensor_copy(z_sb[:], z_ps[:])
                  # [17,1] -T-> [1,17]; zd = (Z' + 15*Nv) * sp; broadcast down
                  # all 128 partitions via K=1 ones-matmul; reciprocal
                  zt_ps = tps.tile([1, 17], f32, tag="tps")
                  nc.tensor.transpose(zt_ps[:], z_sb[:], idf_sb[0:17, 0:17])
                  zt_sb = work.tile([1, 17], f32)
                  nc.vector.tensor_scalar(
                      zt_sb[:],
                      zt_ps[:],
                      znv_sb[0:1, b:b + 1],
                      spv_sb[0:1, 0:1],
                      op0=mybir.AluOpType.add,
                      op1=mybir.AluOpType.mult,
                  )
                  zbc_ps = tps.tile([128, 17], f32, tag="tps")
                  nc.tensor.matmul(
                      zbc_ps[:], onescol_sb[:], zt_sb[:], start=True, stop=True
                  )
                  zinv_sb = work.tile([128, 17], f32)
                  nc.vector.reciprocal(zinv_sb[:, 0:16], zbc_ps[:, 0:16])

                  # --- px'[h, c] (+ u' in row 16): DoubleRow pooling ---------
                  px_sb = work.tile([17, HIDDEN], f32)
                  for c2 in range(2):
                      pp_ps = pxps.tile([17, 512], f32, tag="px")
                      for m2 in range(M2):
                          nc.tensor.matmul(
                              pp_ps[:],
                              e_sb[:, 2 * m2:2 * m2 + 2, 0:17],
                              xp_sb[:, 2 * m2:2 * m2 + 2, c2 * 512:(c2 + 1) * 512],
                              perf_mode=DR,
                              start=(m2 == 0),
                              stop=(m2 == M2 - 1),
                          )
                      nc.scalar.copy(px_sb[:, c2 * 512:(c2 + 1) * 512], pp_ps[:])

                  # --- pxT: [C-tile, h], normalized, b packed in free dim ----
                  for k in range(KT):
                      pxt_ps = tps.tile([128, 17], f32, tag="tps")
                      nc.tensor.transpose(
                          pxt_ps[:], px_sb[:, k * 128:(k + 1) * 128],
                          idf_sb[0:17, 0:17]
                      )
                      tmp_sb = work.tile([128, NH], f32, tag="tmp")
                      nc.vector.tensor_scalar_add(
                          tmp_sb[:], pxt_ps[:, 0:16], pxt_ps[:, 16:17]
                      )
                      nc.vector.tensor_mul(
                          pxall_sb[:, k, :, b], tmp_sb[:], zinv_sb[:, 0:16]
                      )

              # --- stage 3: out1[b, hd] = pxn @ Wv ----------------------------
              out1_ps = bigps.tile([NB, HIDDEN], f32)
              for h in range(NH):
                  for k in range(KT):
                      nc.tensor.matmul(
                          out1_ps[:, h * HD:(h + 1) * HD],
                          pxall_sb[:, k, h, :],
                          wv_sb[:, k, h, :],
                          start=(k == 0),
                          stop=(k == KT - 1),
                      )

              # --- out1T: [hd-tile, b]; per-k so stage 4 chases stage 3 -------
              out1n_sb = small.tile([NB, HIDDEN], f32)
              o1t_sb = small.tile([128, KT, NB], bf16)
              for k in range(KT):
                  nc.vector.tensor_copy(
                      out1n_sb[:, k * 128:(k + 1) * 128],
                      out1_ps[:, k * 128:(k + 1) * 128],
                  )
                  o1t_ps = tps.tile([128, NB], f32, tag="tps")
                  nc.tensor.transpose(
                      o1t_ps[:], out1n_sb[:, k * 128:(k + 1) * 128],
                      idf_sb[0:NB, 0:NB]
                  )
                  nc.vector.tensor_copy(o1t_sb[:, k, :], o1t_ps[:])

              # --- stage 4: out = out1 @ out_w + bias -------------------------
              of_sb = small.tile([NB, PROJ], f32)
              of_ps0 = scps.tile([NB, 512], f32, tag="sc")
              of_ps1 = scps.tile([NB, 512], f32, tag="sc")
              for k in range(KT):
                  for p2, of_ps in ((0, of_ps0), (1, of_ps1)):
                      nc.tensor.matmul(
                          of_ps[:],
                          o1t_sb[:, k, :],
                          wo_sb[:, k, p2, :],
                          start=(k == 0),
                          stop=(k == KT - 1),
                      )
              for p2, of_ps in ((0, of_ps0), (1, of_ps1)):
                  nc.vector.tensor_add(
                      of_sb[:, p2 * 512:(p2 + 1) * 512],
                      of_ps[:],
                      biasrep_sb[:, p2 * 512:(p2 + 1) * 512],
                  )
              nc.sync.dma_start(out_d[:], of_sb[:])

    nc.compile()
    return nc


def _get_nc():
    global _CACHED_NC
    if _CACHED_NC is None:
        _CACHED_NC = _build_nc()
    return _CACHED_NC


def _diffuse_fp8(v):
    """Error-diffusion quantization to fp8 along axis 1 (time)."""
    out = np.empty(v.shape, F8)
    c = np.zeros((v.shape[0], v.shape[2]), np.float32)
    for t in range(v.shape[1]):
        q = (v[:, t, :] + c).astype(F8)
        out[:, t, :] = q
        c = v[:, t, :] + c - q.astype(np.float32)
    return out


def _prep_inputs(hidden_states, mask, kv_w, kv_b, out_w, out_b, query):
    """Host-side sharding + weight preprocessing -> per-core input maps."""
    x = np.ascontiguousarray(hidden_states, dtype=np.float32)
    mask = np.asarray(mask)
    kv_w = np.asarray(kv_w, dtype=np.float32)
    kv_b = np.asarray(kv_b, dtype=np.float32)
    out_w = np.asarray(out_w, dtype=np.float32)
    out_b = np.asarray(out_b, dtype=np.float32)
    query = np.asarray(query, dtype=np.float32)

    scale = 1.0 / HD ** 0.5
    Wk = kv_w[:, :HIDDEN]
    Wv = kv_w[:, HIDDEN:]
    qh = query.reshape(NH, HD)
    # fold query into the k-projection: Wq[c, h]
    Wq = np.einsum("chd,hd->ch", Wk.reshape(HIDDEN, NH, HD), qh) * scale
    bias_final = kv_b[HIDDEN:] @ out_w + out_b  # v-bias is exact post-pool

    # dynamic power-of-2 fp8 scales (exactly unwound inside the exp activation)
    sw = 2.0 ** np.floor(np.log2(F8MAX / max(np.abs(Wq).max(), 1e-30)))
    sx = 2.0 ** np.floor(np.log2(F8MAX / max(np.abs(x).max(), 1e-30)))
    sx = min(sx, 1.0)
    escale = np.full((128, 1), 1.0 / (sw * sx), np.float32)
    wq_r = np.ascontiguousarray(
        (Wq * sw).reshape(KT, 128, NH).transpose(1, 0, 2)
    ).astype(F8)  # [128, KT, NH], fp8 with exp-unwound scale
    wv_r = np.ascontiguousarray(
        Wv.reshape(KT, 128, NH, HD).transpose(1, 0, 2, 3)
    ).astype(BF16)  # [128, KT, NH, HD]
    wo_r = np.ascontiguousarray(
        out_w.reshape(KT, 128, 2, 512).transpose(1, 0, 2, 3)
    ).astype(BF16)  # [128, KT, 2, 512]
    onescol = np.ones((1, 128), np.float32)
    idf = np.eye(128, dtype=np.float32)

    mvalid = (mask != 0).astype(np.float32)      # reference masks where mask == 0
    xm = x * mvalid[:, :, None]                  # pre-masked pooling copy
    sp = 2.0 ** np.floor(np.log2(F8MAX / max(np.abs(xm).max(), 1e-30)))
    # fp8 pooling copy, error-diffused along t, then packed to
    # [B, 128, MT, C] with partition p = t within the 128-chunk
    xp_q = _diffuse_fp8(xm * sp)                 # [B, T, C] fp8
    xp_r = np.ascontiguousarray(
        xp_q.reshape(B, MT, 128, HIDDEN).transpose(0, 2, 1, 3)
    )  # [B, 128, MT, C]

    # xt chunked layout: xtr[b, p, m, k, t] = x[b, m*128+t, k*128+p]
    xt_bf = np.ascontiguousarray(
        (x * sx).reshape(B, MT, 128, KT, 128).transpose(0, 4, 1, 3, 2)
    ).astype(F8)

    nv = mvalid.sum(axis=1)                      # [B] valid counts

    in_maps = []
    for c in range(NCORES):
        sl = slice(c * NB, (c + 1) * NB)
        # mcol[p, b, m, 0] = valid(mask[b, m*128+p]); padded to 16 cols
        mcol = np.zeros((128, NB, MT, 16), F8)
        mcol[:, :, :, 0] = mvalid[sl].reshape(NB, MT, 128).transpose(2, 0, 1).astype(F8)
        in_maps.append({
            "xt": xt_bf[sl],
            "xp": xp_r[sl],
            "wq": wq_r,
            "wv": wv_r,
            "wo": wo_r,
            "mcol": mcol,
            "biasrep": np.ascontiguousarray(
                np.broadcast_to(bias_final[None, :], (NB, PROJ))
            ),
            "onescol": onescol,
            "idf": idf,
            "escale": escale,
            "ebias": np.full((128, 1), LN16, np.float32),
            "znv": np.ascontiguousarray(15.0 * nv[sl][None, :].astype(np.float32)),
            "spv": np.full((1, 1), sp, np.float32),
        })
    return in_maps


def kernel(hidden_states, mask, kv_w, kv_b, out_w, out_b, query, **_unused):
    from concourse.bass_utils import run_bass_kernel_spmd

    nc = _get_nc()
    in_maps = _prep_inputs(hidden_states, mask, kv_w, kv_b, out_w, out_b, query)
    res = run_bass_kernel_spmd(nc, in_maps, list(range(NCORES)))
    out = np.concatenate([res.results[i]["out"] for i in range(NCORES)], axis=0)
    return out.astype(np.float32)


# revision 12
# speedup vs baseline: 1.2346x; 1.0676x over previous
"""AudioAttentionPooler Trainium2 kernel (v2).

Algorithm (algebraically identical to the reference, ~60x fewer FLOPs):
  scores[b,t,h] = x[b,t,:] @ Wq[:,h]        Wq = fold(query*scale, kv_w_k)  [C,h]
  (k-bias shifts scores uniformly along t -> softmax-invariant -> dropped)
  e'' = 16*exp(scores)  (exp input-scale unwinds the fp8 scales)
  e'  = e'' - 15        (fp8; w = (15 + e')/16 up to the global 1/16Z norm)
  Z16[b,h] = sum_t (15 + e'[b,t,h]) * mask[b,t]       (= 16*Z)
  px'[b,h,:] = sum_t e'[b,t,h] * xp[b,t,:]            (pool BEFORE v-proj)
  u'[b,:]    = sum_t 15 * xp[b,t,:]                   (u-column of the same matmul)
  pxn[b,h,:] = (px' + u') / (Z16 * sp)                (xp carries scale sp)
  out1[b,h*64+d] = pxn[b,h,:] @ Wv[:,h*64+d]
  out = out1 @ out_w + (kv_b_v @ out_w + out_b)   (v-bias exact: attn sums to 1)

v2 changes vs v1:
  - pooling copy of x is fp8 with host-side error diffusion along t
    (quantization carries cancel in the near-uniform attention sum),
    halving the dominant DMA stream (16.8MB bf16 -> 8.4MB fp8 per core)
  - pooling matmuls use fp8 DoubleRow perf mode (2 t-tiles per
    instruction, 2x moving-operand rate), with the attention weights as
    the 17-column stationary (16 heads + constant-15 u column)
  - Z comes from tiny DoubleRow matmuls against a mask column

Sharding: data-parallel over batch, 4 batch elements per core x 8 cores.
"""

import numpy as np
import ml_dtypes

BF16 = ml_dtypes.bfloat16
F8 = ml_dtypes.float8_e4m3

HIDDEN = 1024
NH = 16
HD = 64
PROJ = 1024
B, T = 32, 2048
NCORES = 8
NB = B // NCORES          # 4 batch elems per core
KT = HIDDEN // 128        # 8 C-tiles
MT = T // 128             # 16 T-chunks
M2 = MT // 2              # 8 DoubleRow pairs
F8MAX = 240.0             # conservative e4m3 range cap
LN16 = float(np.log(16.0))

_CACHED_NC = None


def _build_nc(reps=1):
    import concourse.bacc as bacc
    import concourse.mybir as mybir
    import concourse.tile as tile

    f32 = mybir.dt.float32
    bf16 = mybir.dt.bfloat16
    f8 = mybir.dt.float8e4
    DR = mybir.MatmulPerfMode.DoubleRow

    nc = bacc.Bacc("TRN2", target_bir_lowering=False, debug=False)

    xt_d = nc.dram_tensor("xt", [NB, 128, MT, KT, 128], f8, kind="ExternalInput")
    xp_d = nc.dram_tensor("xp", [NB, 128, MT, HIDDEN], f8, kind="ExternalInput")
    wq_d = nc.dram_tensor("wq", [128, KT, NH], f8, kind="ExternalInput")
    wv_d = nc.dram_tensor("wv", [128, KT, NH, HD], bf16, kind="ExternalInput")
    wo_d = nc.dram_tensor("wo", [128, KT, 2, 512], bf16, kind="ExternalInput")
    mcol_d = nc.dram_tensor("mcol", [128, NB, MT, 16], f8, kind="ExternalInput")
    biasrep_d = nc.dram_tensor("biasrep", [NB, PROJ], f32, kind="ExternalInput")
    onescol_d = nc.dram_tensor("onescol", [1, 128], f32, kind="ExternalInput")
    idf_d = nc.dram_tensor("idf", [128, 128], f32, kind="ExternalInput")
    escale_d = nc.dram_tensor("escale", [128, 1], f32, kind="ExternalInput")
    ebias_d = nc.dram_tensor("ebias", [128, 1], f32, kind="ExternalInput")
    znv_d = nc.dram_tensor("znv", [1, NB], f32, kind="ExternalInput")
    spv_d = nc.dram_tensor("spv", [1, 1], f32, kind="ExternalInput")
    out_d = nc.dram_tensor("out", [NB, PROJ], f32, kind="ExternalOutput")

    from contextlib import nullcontext

    with tile.TileContext(nc) as tc:
        with (
            tc.tile_pool(name="consts", bufs=1) as consts,
            tc.tile_pool(name="xppool", bufs=3) as xppool,
            tc.tile_pool(name="xtpool", bufs=3) as xtpool,
            tc.tile_pool(name="work", bufs=3) as work,
            tc.tile_pool(name="small", bufs=1) as small,
            tc.tile_pool(name="scps", bufs=2, space="PSUM") as scps,
            tc.tile_pool(name="pxps", bufs=2, space="PSUM") as pxps,
            tc.tile_pool(name="tps", bufs=1, space="PSUM") as tps,
            tc.tile_pool(name="bigps", bufs=1, space="PSUM") as bigps,
        ):
            wq_sb = consts.tile([128, KT, NH], f8)
            wv_sb = consts.tile([128, KT, NH, HD], bf16)
            wo_sb = consts.tile([128, KT, 2, 512], bf16)
            mcol_sb = consts.tile([128, NB, MT, 16], f8)
            biasrep_sb = consts.tile([NB, PROJ], f32)
            onescol_sb = consts.tile([1, 128], f32)
            idf_sb = consts.tile([128, 128], f32)
            escale_sb = consts.tile([128, 1], f32)
            ebias_sb = consts.tile([128, 1], f32)
            znv_sb = consts.tile([1, NB], f32)
            spv_sb = consts.tile([1, 1], f32)
            nc.sync.dma_start(wq_sb[:], wq_d[:])

            # persistent accumulators across the b-loop
            pxall_sb = small.tile([128, KT, NH, NB], bf16)

            rep_ctx = tc.For_i(0, reps, 1) if reps > 1 else nullcontext()
            with rep_ctx:
              # Software pipeline: scores(b) || pooling(b-1), so the PE never
              # waits for the xp stream (pooling data arrived one slot ago).
              prev = None
              for b in range(NB + 1):
                cur = None
                if b < NB:
                  xt_sb = xtpool.tile([128, MT, KT, 128], f8)
                  xp_sb = xppool.tile([128, MT, HIDDEN], f8)
                  for m4 in range(4):
                      nc.sync.dma_start(
                          xt_sb[:, m4 * 4:(m4 + 1) * 4],
                          xt_d[b, :, m4 * 4:(m4 + 1) * 4],
                      )
                  for m4 in range(4):
                      nc.sync.dma_start(
                          xp_sb[:, m4 * 4:(m4 + 1) * 4],
                          xp_d[b, :, m4 * 4:(m4 + 1) * 4],
                      )
                  # deferred const loads: small consts after the first batch
                  # data stream, big stage-3/4 weights after ALL batch data
                  if b == 0:
                      nc.sync.dma_start(escale_sb[:], escale_d[:])
                      nc.sync.dma_start(ebias_sb[:], ebias_d[:])
                      nc.sync.dma_start(onescol_sb[:], onescol_d[:])
                      nc.sync.dma_start(znv_sb[:], znv_d[:])
                      nc.sync.dma_start(spv_sb[:], spv_d[:])
                      nc.sync.dma_start(mcol_sb[:], mcol_d[:])
                      nc.sync.dma_start(idf_sb[:], idf_d[:])
                  elif b == NB - 1:
                      for k in range(KT):
                          nc.sync.dma_start(wv_sb[:, k], wv_d[:, k])
                      for k in range(KT):
                          nc.sync.dma_start(wo_sb[:, k], wo_d[:, k])
                      nc.sync.dma_start(biasrep_sb[:], biasrep_d[:])

                  # --- scores[t, h] = x @ Wq; e' = 16*exp(s) - 15 (fp8) ------
                  e_sb = work.tile([128, MT, 32], f8)
                  nc.vector.memset(e_sb[:, :, 16:17], 15.0)
                  for m2 in range(MT // 4):
                      sc_ps = scps.tile([128, 4, NH], f32, tag="sc")
                      for m4 in range(4):
                          m = m2 * 4 + m4
                          for k in range(KT):
                              nc.tensor.matmul(
                                  sc_ps[:, m4, :],
                                  xt_sb[:, m, k, :],
                                  wq_sb[:, k, :],
                                  start=(k == 0),
                                  stop=(k == KT - 1),
                              )
                      ebig_sb = work.tile([128, 4, NH], f32, tag="ebig")
                      nc.scalar.activation(
                          ebig_sb[:],
                          sc_ps[:],
                          mybir.ActivationFunctionType.Exp,
                          bias=ebias_sb[:],
                          scale=escale_sb[:],
                      )
                      nc.vector.tensor_scalar_sub(
                          e_sb[:, m2 * 4:(m2 + 1) * 4, 0:16], ebig_sb[:], 15.0
                      )

                  # --- Z16[h] = sum_t (15 + e') * mask (transposed: [1, 17]);
                  # zinv = 1/((Z' + 15*Nv) * sp); broadcast down 128
                  # partitions via a K=1 ones-matmul ---------------------------
                  z_ps = tps.tile([1, 32], f32, tag="zps")
                  for m2 in range(M2):
                      nc.tensor.matmul(
                          z_ps[0:1, 0:17],
                          mcol_sb[:, b, 2 * m2:2 * m2 + 2, 0:1],
                          e_sb[:, 2 * m2:2 * m2 + 2, 0:17],
                          perf_mode=DR,
                          start=(m2 == 0),
                          stop=(m2 == M2 - 1),
                      )
                  zaff_sb = work.tile([1, 17], f32, tag="zaff")
                  nc.vector.tensor_scalar(
                      zaff_sb[:],
                      z_ps[0:1, 0:17],
                      znv_sb[0:1, b:b + 1],
                      spv_sb[0:1, 0:1],
                      op0=mybir.AluOpType.add,
                      op1=mybir.AluOpType.mult,
                  )
                  zinv1_sb = work.tile([1, 17], f32, tag="zinv1")
                  nc.vector.reciprocal(zinv1_sb[0:1, 0:16], zaff_sb[0:1, 0:16])
                  zbc_ps = tps.tile([128, 32], f32, tag="zps")
                  nc.tensor.matmul(
                      zbc_ps[:, 0:16], onescol_sb[:], zinv1_sb[0:1, 0:16],
                      start=True, stop=True
                  )
                  zinv_sb = work.tile([128, NH], f32, tag="zinv")
                  nc.vector.tensor_copy(zinv_sb[:], zbc_ps[:, 0:16])
                  cur = (b, e_sb, xp_sb, zinv_sb)

                if prev is not None:
                  pb, e_p, xp_p, zinv_p = prev
                  # --- pooling: pxn-oriented [c, 17] per k; xp is the
                  # stationary so the output needs no transpose --------------
                  pall_sb = work.tile([128, KT, 32], f32, tag="pall")
                  for k in range(KT):
                      pk_ps = pxps.tile([128, 32], f32, tag="px")
                      for m in range(MT):
                          nc.tensor.matmul(
                              pk_ps[:, 0:17],
                              xp_p[:, m, k * 128:(k + 1) * 128],
                              e_p[:, m, 0:17],
                              start=(m == 0),
                              stop=(m == MT - 1),
                          )
                      nc.scalar.copy(pall_sb[:, k], pk_ps[:])
                  # --- merge: pxn[c, h] = (px' + u') * zinv ------------------
                  for k in range(KT):
                      tmp_sb = work.tile([128, NH], f32, tag="tmp")
                      nc.vector.tensor_scalar_add(
                          tmp_sb[:], pall_sb[:, k, 0:16], pall_sb[:, k, 16:17]
                      )
                      nc.vector.tensor_mul(
                          pxall_sb[:, k, :, pb], tmp_sb[:], zinv_p[:]
                      )
                prev = cur

              # --- stage 3: out1[b, hd] = pxn @ Wv; k-outer so the matmuls
              # chase the per-k wv DMA chunks ----------------------------------
              out1_ps = bigps.tile([NB, HIDDEN], f32)
              for h in range(NH):
                  for k in range(KT):
                      nc.tensor.matmul(
                          out1_ps[:, h * HD:(h + 1) * HD],
                          pxall_sb[:, k, h, :],
                          wv_sb[:, k, h, :],
                          start=(k == 0),
                          stop=(k == KT - 1),
                      )

              # --- out1T: [hd-tile, b]; per-k so stage 4 chases stage 3 -------
              out1n_sb = small.tile([NB, HIDDEN], f32)
              o1t_sb = small.tile([128, KT, NB], bf16)
              for k in range(KT):
                  nc.vector.tensor_copy(
                      out1n_sb[:, k * 128:(k + 1) * 128],
                      out1_ps[:, k * 128:(k + 1) * 128],
                  )
                  o1t_t = tps.tile([128, 64], f32, tag="tps")
                  o1t_ps = o1t_t[:, 0:NB]
                  nc.tensor.transpose(
                      o1t_ps[:], out1n_sb[:, k * 128:(k + 1) * 128],
                      idf_sb[0:NB, 0:NB]
                  )
                  nc.vector.tensor_copy(o1t_sb[:, k, :], o1t_ps[:])

              # --- stage 4: out = out1 @ out_w + bias -------------------------
              of_sb = small.tile([NB, PROJ], f32)
              of_ps0 = scps.tile([NB, 512], f32, tag="sc")
              of_ps1 = scps.tile([NB, 512], f32, tag="sc")
              for k in range(KT):
                  for p2, of_ps in ((0, of_ps0), (1, of_ps1)):
                      nc.tensor.matmul(
                          of_ps[:],
                          o1t_sb[:, k, :],
                          wo_sb[:, k, p2, :],
                          start=(k == 0),
                          stop=(k == KT - 1),
                      )
              for p2, of_ps in ((0, of_ps0), (1, of_ps1)):
                  nc.vector.tensor_add(
                      of_sb[:, p2 * 512:(p2 + 1) * 512],
                      of_ps[:],
                      biasrep_sb[:, p2 * 512:(p2 + 1) * 512],
                  )
              nc.sync.dma_start(out_d[:], of_sb[:])

    nc.compile()
    return nc


def _get_nc():
    global _CACHED_NC
    if _CACHED_NC is None:
        _CACHED_NC = _build_nc()
    return _CACHED_NC


def _diffuse_fp8(v):
    """Error-diffusion quantization to fp8 along axis 1 (time)."""
    out = np.empty(v.shape, F8)
    c = np.zeros((v.shape[0], v.shape[2]), np.float32)
    for t in range(v.shape[1]):
        q = (v[:, t, :] + c).astype(F8)
        out[:, t, :] = q
        c = v[:, t, :] + c - q.astype(np.float32)
    return out


def _prep_inputs(hidden_states, mask, kv_w, kv_b, out_w, out_b, query):
    """Host-side sharding + weight preprocessing -> per-core input maps."""
    x = np.ascontiguousarray(hidden_states, dtype=np.float32)
    mask = np.asarray(mask)
    kv_w = np.asarray(kv_w, dtype=np.float32)
    kv_b = np.asarray(kv_b, dtype=np.float32)
    out_w = np.asarray(out_w, dtype=np.float32)
    out_b = np.asarray(out_b, dtype=np.float32)
    query = np.asarray(query, dtype=np.float32)

    scale = 1.0 / HD ** 0.5
    Wk = kv_w[:, :HIDDEN]
    Wv = kv_w[:, HIDDEN:]
    qh = query.reshape(NH, HD)
    # fold query into the k-projection: Wq[c, h]
    Wq = np.einsum("chd,hd->ch", Wk.reshape(HIDDEN, NH, HD), qh) * scale
    bias_final = kv_b[HIDDEN:] @ out_w + out_b  # v-bias is exact post-pool

    # dynamic power-of-2 fp8 scales (exactly unwound inside the exp activation)
    sw = 2.0 ** np.floor(np.log2(F8MAX / max(np.abs(Wq).max(), 1e-30)))
    sx = 2.0 ** np.floor(np.log2(F8MAX / max(np.abs(x).max(), 1e-30)))
    sx = min(sx, 1.0)
    escale = np.full((128, 1), 1.0 / (sw * sx), np.float32)
    wq_r = np.ascontiguousarray(
        (Wq * sw).reshape(KT, 128, NH).transpose(1, 0, 2)
    ).astype(F8)  # [128, KT, NH], fp8 with exp-unwound scale
    wv_r = np.ascontiguousarray(
        Wv.reshape(KT, 128, NH, HD).transpose(1, 0, 2, 3)
    ).astype(BF16)  # [128, KT, NH, HD]
    wo_r = np.ascontiguousarray(
        out_w.reshape(KT, 128, 2, 512).transpose(1, 0, 2, 3)
    ).astype(BF16)  # [128, KT, 2, 512]
    onescol = np.ones((1, 128), np.float32)
    idf = np.eye(128, dtype=np.float32)

    mvalid = (mask != 0).astype(np.float32)      # reference masks where mask == 0
    xm = x * mvalid[:, :, None]                  # pre-masked pooling copy
    sp = 2.0 ** np.floor(np.log2(F8MAX / max(np.abs(xm).max(), 1e-30)))
    # fp8 pooling copy, error-diffused along t, then packed to
    # [B, 128, MT, C] with partition p = t within the 128-chunk
    xp_q = _diffuse_fp8(xm * sp)                 # [B, T, C] fp8
    xp_r = np.ascontiguousarray(
        xp_q.reshape(B, MT, 128, HIDDEN).transpose(0, 2, 1, 3)
    )  # [B, 128, MT, C]

    # xt chunked layout: xtr[b, p, m, k, t] = x[b, m*128+t, k*128+p]
    xt_bf = np.ascontiguousarray(
        (x * sx).reshape(B, MT, 128, KT, 128).transpose(0, 4, 1, 3, 2)
    ).astype(F8)

    nv = mvalid.sum(axis=1)                      # [B] valid counts

    in_maps = []
    for c in range(NCORES):
        sl = slice(c * NB, (c + 1) * NB)
        # mcol[p, b, m, 0] = valid(mask[b, m*128+p]); padded to 16 cols
        mcol = np.zeros((128, NB, MT, 16), F8)
        mcol[:, :, :, 0] = mvalid[sl].reshape(NB, MT, 128).transpose(2, 0, 1).astype(F8)
        in_maps.append({
            "xt": xt_bf[sl],
            "xp": xp_r[sl],
            "wq": wq_r,
            "wv": wv_r,
            "wo": wo_r,
            "mcol": mcol,
            "biasrep": np.ascontiguousarray(
                np.broadcast_to(bias_final[None, :], (NB, PROJ))
            ),
            "onescol": onescol,
            "idf": idf,
            "escale": escale,
            "ebias": np.full((128, 1), LN16, np.float32),
            "znv": np.ascontiguousarray(15.0 * nv[sl][None, :].astype(np.float32)),
            "spv": np.full((1, 1), sp, np.float32),
        })
    return in_maps


def kernel(hidden_states, mask, kv_w, kv_b, out_w, out_b, query, **_unused):
    from concourse.bass_utils import run_bass_kernel_spmd

    nc = _get_nc()
    in_maps = _prep_inputs(hidden_states, mask, kv_w, kv_b, out_w, out_b, query)
    res = run_bass_kernel_spmd(nc, in_maps, list(range(NCORES)))
    out = np.concatenate([res.results[i]["out"] for i in range(NCORES)], axis=0)
    return out.astype(np.float32)
